# revision 9
# baseline (speedup 1.0000x reference)
"""Trainium2 Bass kernel for a dense attention block.

Reference computation (per batch b, head h):
    att = (q @ k^T) / sqrt(D) + att_mask          # [S, S]
    att = where(padding_mask[b], -inf, att)
    out = softmax(att, -1) @ v                    # [S, D]

Shapes: q,k,v [4, 16, 2048, 64] f32; att_mask [1,1,2048,2048] f32;
padding_mask [4, 2048, 2048] bool.  Output [4, 16, 2048, 64] f32.

Sharding over 8 cores: core c handles batch b=c//2, heads h in
[8*(c%2), 8*(c%2)+8).  Each core computes 8 full attention heads.

Device algorithm (per core), transposed-score formulation so that the
softmax reduction lands on the PE contraction axis:
  - W^T[k,q] = exp(att_mask[q,k]) * (1 - pad[q,k])   (fp16, SBUF-resident)
    softmax(s+m) == exp(s)*exp(m) / sum(exp(s)*exp(m)); masked entries
    multiply to exactly 0.  No max-subtraction is needed: |scores| <= ~10
    for this distribution, well within fp32/fp16 exp range.
  - per head, per 1024-wide q-block, per 128-wide k-chunk j:
      S^T_j [128k, 1024q] = K_j @ Q^T   (fp32r matmuls, PSUM)
      E_j   = exp(S^T_j / 8)            (ACT, fp16 out)
      EW_j  = E_j * W^T_j               (DVE fp16 2x)
      O^T  += V'_j^T @ EW_j             (fp16 matmul, V' has a ones column
                                         so row 64 of O^T is the denominator)
  - transpose O^T back with PE, multiply by 1/Z, DMA out.
"""

import sys

if "/opt/trn_rl_repo" not in sys.path:
    sys.path.insert(0, "/opt/trn_rl_repo")

import numpy as np

import concourse.bass as bass
import concourse.tile as tile
from concourse import bacc, mybir
from concourse.bass import ts
from concourse.bass_utils import run_bass_kernel_spmd
from concourse.masks import make_identity

F32 = mybir.dt.float32
F32R = mybir.dt.float32r
F16 = mybir.dt.float16
U8 = mybir.dt.uint8

B, H, S, D = 4, 16, 2048, 64
N_CORES = 8
HPC = H // 2          # heads per core
KC = 128              # k-chunk (PSUM partition dim of S^T)
NKC = S // KC         # 16 k-chunks
QB = 1024             # q-block
NQB = S // QB         # q-blocks per head
MM_N = 512            # moving-operand cols per fp32r matmul
SCALE = 1.0 / np.sqrt(D)


def build_program():
    """Build the per-core Bass program (SPMD: identical on all 8 cores)."""
    nc = bacc.Bacc("TRN2", target_bir_lowering=False, debug=False,
                   num_devices=N_CORES)

    qT = nc.declare_dram_parameter("qT", [HPC, D, S], F32R, isOutput=False)
    kT = nc.declare_dram_parameter("kT", [HPC, D, S], F32R, isOutput=False)
    v_ = nc.declare_dram_parameter("v", [HPC, S, D], F32, isOutput=False)
    attT = nc.declare_dram_parameter("attT", [S, S], F32, isOutput=False)
    padT = nc.declare_dram_parameter("padT", [S, S], U8, isOutput=False)
    out = nc.declare_dram_parameter("out", [HPC, S, D], F32, isOutput=True)

    with tile.TileContext(nc, num_cores=N_CORES) as tc:
        with (
            tc.tile_pool(name="singles", bufs=1) as singles,
            tc.tile_pool(name="wprep", bufs=2) as wprep,
            tc.tile_pool(name="heads", bufs=2) as heads,
            tc.tile_pool(name="chunks", bufs=3) as chunks,
            tc.tile_pool(name="outs", bufs=2) as outs,
            tc.tile_pool(name="sp", bufs=2, space="PSUM") as sp_pool,
            tc.tile_pool(name="op", bufs=1, space="PSUM") as op_pool,
            tc.tile_pool(name="otp", bufs=1, space="PSUM") as ot_pool,
        ):
            # ---- constants ----
            ident = singles.tile([128, 128], F32, tag="ident")
            make_identity(nc, ident[:])
            # V' layout [128, NKC, 65] fp16: col 64 of each chunk group = 1.0
            vp = singles.tile([128, NKC, D + 1], F16, tag="vp")
            nc.gpsimd.memset(vp[:, :, D], 1.0)

            # ---- W^T = exp(attT) * (1 - padT), fp16, SBUF resident ----
            wt = [singles.tile([128, S], F16, name=f"w{j}", tag=f"w{j}")
                  for j in range(NKC)]
            for j in range(NKC):
                att_blk = wprep.tile([128, S], F32, tag="att_blk")
                nc.gpsimd.dma_start(att_blk[:], attT[ts(j, 128), :])
                pad_blk = wprep.tile([128, S], U8, tag="pad_blk")
                nc.gpsimd.dma_start(pad_blk[:], padT[ts(j, 128), :])
                expat = wprep.tile([128, S], F16, tag="expat")
                nc.scalar.activation(expat[:], att_blk[:],
                                     mybir.ActivationFunctionType.Exp)
                # (1 - pad) as fp16 via DVE tensor_scalar (u8 -> f16 convert)
                padf = wprep.tile([128, S], F16, tag="padf")
                nc.vector.tensor_scalar(padf[:], pad_blk[:], -1.0, 1.0,
                                        mybir.AluOpType.mult,
                                        mybir.AluOpType.add)
                nc.vector.tensor_mul(wt[j][:], expat[:], padf[:])

            # ---- main loop over heads ----
            for h in range(HPC):
                kt_h = heads.tile([D, S], F32R, tag="kt")
                nc.gpsimd.dma_start(kt_h[:], kT[h])
                qt_h = heads.tile([D, S], F32R, tag="qt")
                nc.gpsimd.dma_start(qt_h[:], qT[h])
                v32 = heads.tile([128, NKC, D], F32, tag="v32")
                nc.gpsimd.dma_start(
                    v32[:], v_[h].rearrange("(c p) d -> p c d", p=128))
                # cast V into fp16 V' (ones columns preset)
                nc.vector.tensor_copy(vp[:, :, 0:D], v32[:])

                for qb in range(NQB):
                    o_ps = op_pool.tile([D + 1, QB], F32, tag="op")
                    for j in range(NKC):
                        s_ps = sp_pool.tile([128, QB], F32, tag="sp")
                        for m in range(QB // MM_N):
                            nc.tensor.matmul(
                                s_ps[:, ts(m, MM_N)],
                                lhsT=kt_h[:, ts(j, 128)],
                                rhs=qt_h[:, qb * QB + m * MM_N:
                                         qb * QB + (m + 1) * MM_N],
                                start=True, stop=True)
                        e16 = chunks.tile([128, QB], F16, tag="e16")
                        nc.scalar.activation(e16[:], s_ps[:],
                                             mybir.ActivationFunctionType.Exp,
                                             scale=float(SCALE))
                        ew = chunks.tile([128, QB], F16, tag="ew")
                        nc.vector.tensor_mul(ew[:], e16[:],
                                             wt[j][:, qb * QB:(qb + 1) * QB])
                        for m in range(QB // MM_N):
                            nc.tensor.matmul(o_ps[:, ts(m, MM_N)],
                                             lhsT=vp[:, j, :],
                                             rhs=ew[:, ts(m, MM_N)],
                                             start=(j == 0),
                                             stop=(j == NKC - 1))

                    # ---- normalize + transpose + store ----
                    o_sb = outs.tile([D + 1, QB], F32, tag="o_sb")
                    nc.vector.tensor_copy(o_sb[:], o_ps[:])
                    ot = ot_pool.tile([128, QB // 128, D], F32, tag="ot")
                    otz = ot_pool.tile([128, QB // 128], F32, tag="otz")
                    for t in range(QB // 128):
                        nc.tensor.transpose(ot[:, t, :], o_sb[0:D, ts(t, 128)],
                                            ident[0:D, 0:D])
                        nc.tensor.transpose(otz[:, t:t + 1],
                                            o_sb[D:D + 1, ts(t, 128)],
                                            ident[D:D + 1, D:D + 1])
                    rz = outs.tile([128, QB // 128], F32, tag="rz")
                    nc.vector.reciprocal(rz[:], otz[:])
                    o_st = outs.tile([128, QB // 128, D], F32, tag="o_st")
                    nc.vector.tensor_mul(
                        o_st[:], ot[:],
                        rz[:].broadcast_to((128, QB // 128, D)))
                    nc.gpsimd.dma_start(
                        out[h, qb * QB:(qb + 1) * QB, :].rearrange(
                            "(t p) d -> p t d", p=128),
                        o_st[:])
    nc.finalize()
    return nc


_CACHED_NC = None


def _get_program():
    global _CACHED_NC
    if _CACHED_NC is None:
        _CACHED_NC = build_program()
    return _CACHED_NC


def _round_tf32(x):
    """Round fp32 to the fp32r (TF32, 10-bit mantissa) grid, to nearest.

    The PE consumes fp32r operands pre-rounded (the BIR verifier enforces a
    float32r producer); rounding on host costs nothing on device."""
    xi = x.view(np.int32)
    return ((xi + 0x1000) & ~0x1FFF).astype(np.int32).view(np.float32)


def shard_inputs(q, k, v, att_mask, padding_mask):
    """Host-side sharding + layout transforms (transposes only, no math)."""
    attT = np.ascontiguousarray(att_mask[0, 0].T)
    padT = [np.ascontiguousarray(padding_mask[b].T).view(np.uint8)
            for b in range(B)]
    in_maps = []
    for c in range(N_CORES):
        b, hh = divmod(c, 2)
        h0 = hh * HPC
        qc = q[b, h0:h0 + HPC]
        kc = k[b, h0:h0 + HPC]
        in_maps.append({
            "qT": _round_tf32(np.ascontiguousarray(qc.transpose(0, 2, 1))),
            "kT": _round_tf32(np.ascontiguousarray(kc.transpose(0, 2, 1))),
            "v": np.ascontiguousarray(v[b, h0:h0 + HPC]),
            "attT": attT,
            "padT": padT[b],
        })
    return in_maps


def unshard_output(results):
    out = np.empty((B, H, S, D), dtype=np.float32)
    for c in range(N_CORES):
        b, hh = divmod(c, 2)
        h0 = hh * HPC
        out[b, h0:h0 + HPC] = results[c]["out"]
    return out


def kernel(q, k, v, att_mask, padding_mask):
    q = np.asarray(q, dtype=np.float32)
    k = np.asarray(k, dtype=np.float32)
    v = np.asarray(v, dtype=np.float32)
    att_mask = np.asarray(att_mask, dtype=np.float32)
    padding_mask = np.asarray(padding_mask)
    nc = _get_program()
    in_maps = shard_inputs(q, k, v, att_mask, padding_mask)
    res = run_bass_kernel_spmd(nc, in_maps, list(range(N_CORES)))
    return unshard_output(res.results)


# revision 15
# speedup vs baseline: 3.0057x; 3.0057x over previous
"""Trainium2 Bass kernel for a dense attention block.

Reference computation (per batch b, head h):
    att = (q @ k^T) / sqrt(D) + att_mask          # [S, S]
    att = where(padding_mask[b], -inf, att)
    out = softmax(att, -1) @ v                    # [S, D]

Shapes: q,k,v [4, 16, 2048, 64] f32; att_mask [1,1,2048,2048] f32;
padding_mask [4, 2048, 2048] bool.  Output [4, 16, 2048, 64] f32.

Sharding over 8 cores: core c handles batch b=c//2, heads h in
[8*(c%2), 8*(c%2)+8).  Each core computes 8 full attention heads.

Device algorithm (per core), transposed-score formulation so that the
softmax reduction lands on the PE contraction axis:
  - W^T[k,q] = exp(att_mask[q,k]) * (1 - pad[q,k])   (fp16, SBUF-resident)
    softmax(s+m) == exp(s)*exp(m) / sum(exp(s)*exp(m)); masked entries
    multiply to exactly 0.  No max-subtraction is needed: |scores| <= ~10
    for this distribution, well within fp32/fp16 exp range.
  - per head, per 1024-wide q-block, per 128-wide k-chunk j:
      S^T_j [128k, 1024q] = K_j @ Q^T   (fp32r matmuls, PSUM)
      E_j   = exp(S^T_j / 8)            (ACT, fp16 out)
      EW_j  = E_j * W^T_j               (DVE fp16 2x)
      O^T  += V'_j^T @ EW_j             (fp16 matmul, V' has a ones column
                                         so row 64 of O^T is the denominator)
  - transpose O^T back with PE, multiply by 1/Z, DMA out.
"""

import sys

if "/opt/trn_rl_repo" not in sys.path:
    sys.path.insert(0, "/opt/trn_rl_repo")

import numpy as np

import concourse.bass as bass
import concourse.tile as tile
from concourse import bacc, mybir
from concourse.bass import ts
from concourse.bass_utils import run_bass_kernel_spmd
from concourse.masks import make_identity

F32 = mybir.dt.float32
F32R = mybir.dt.float32r
F16 = mybir.dt.float16
U8 = mybir.dt.uint8

B, H, S, D = 4, 16, 2048, 64
N_CORES = 8
HPC = H // 2          # heads per core
KC = 128              # k-chunk (PSUM partition dim of S^T)
NKC = S // KC         # 16 k-chunks
QB = 1024             # q-block
NQB = S // QB         # q-blocks per head
MM_N = 512            # moving-operand cols per fp32r matmul
SCALE = 1.0 / np.sqrt(D)


def build_program(n_heads=HPC, repeat=1, stage='full'):
    """Build the per-core Bass program (SPMD: identical on all 8 cores).

    repeat>1 re-runs the head loop (timing aid: the device-side cost of one
    pass equals the per-repeat time delta, independent of dispatch latency).
    """
    nc = bacc.Bacc("TRN2", target_bir_lowering=False, debug=False,
                   num_devices=N_CORES)

    qT = nc.declare_dram_parameter("qT", [HPC, D, S], F32, isOutput=False)
    kT = nc.declare_dram_parameter("kT", [HPC, D, S], F32, isOutput=False)
    v_ = nc.declare_dram_parameter("v", [HPC, S, D], F32, isOutput=False)
    attT = nc.declare_dram_parameter("attT", [S, S], F32, isOutput=False)
    padT = nc.declare_dram_parameter("padT", [S, S], U8, isOutput=False)
    out = nc.declare_dram_parameter("out", [HPC, S, D], F32, isOutput=True)

    with tile.TileContext(nc, num_cores=N_CORES) as tc:
        with (
            tc.tile_pool(name="singles", bufs=1) as singles,
            tc.tile_pool(name="wprep", bufs=2) as wprep,
            tc.tile_pool(name="heads", bufs=2) as heads,
            tc.tile_pool(name="chunks", bufs=3) as chunks,
            tc.tile_pool(name="outs", bufs=2) as outs,
            tc.tile_pool(name="sp", bufs=2, space="PSUM") as sp_pool,
            tc.tile_pool(name="op", bufs=1, space="PSUM") as op_pool,
            tc.tile_pool(name="otp", bufs=1, space="PSUM") as ot_pool,
        ):
            # ---- constants ----
            ident = singles.tile([128, 128], F32, tag="ident")
            make_identity(nc, ident[:])
            # V' layout [128, NKC, 65] fp16: col 64 of each chunk group = 1.0
            vp = singles.tile([128, NKC, D + 1], F16, tag="vp")
            nc.gpsimd.memset(vp[:, :, D], 1.0)

            # ---- W^T = exp(attT) * (1 - padT), fp16, SBUF resident ----
            wt = [singles.tile([128, S], F16, name=f"w{j}", tag=f"w{j}")
                  for j in range(NKC)]
            for j in range(NKC):
                att_blk = wprep.tile([128, S], F32, tag="att_blk")
                nc.gpsimd.dma_start(att_blk[:], attT[ts(j, 128), :])
                pad_blk = wprep.tile([128, S], U8, tag="pad_blk")
                nc.gpsimd.dma_start(pad_blk[:], padT[ts(j, 128), :])
                expat = wprep.tile([128, S], F16, tag="expat")
                nc.scalar.activation(expat[:], att_blk[:],
                                     mybir.ActivationFunctionType.Exp)
                # (1 - pad) as fp16 via DVE tensor_scalar (u8 -> f16 convert)
                padf = wprep.tile([128, S], F16, tag="padf")
                nc.vector.tensor_scalar(padf[:], pad_blk[:], -1.0, 1.0,
                                        mybir.AluOpType.mult,
                                        mybir.AluOpType.add)
                nc.vector.tensor_mul(wt[j][:], expat[:], padf[:])

            # ---- main loop over heads ----
            for h_rep in range(n_heads * repeat):
                h = h_rep % n_heads
                kt_h = heads.tile([D, S], F16, tag="kt")
                nc.gpsimd.dma_start(kt_h[:], kT[h])
                qt_h = heads.tile([D, S], F16, tag="qt")
                nc.gpsimd.dma_start(qt_h[:], qT[h])
                nc.gpsimd.dma_start(
                    vp[:, :, 0:D], v_[h].rearrange("(c p) d -> p c d", p=128))

                for qb in range(NQB):
                    o_ps = op_pool.tile([D + 1, QB], F32, tag="op")
                    for j in range(NKC):
                        s_ps = sp_pool.tile([128, QB], F32, tag="sp")
                        for m in range(QB // MM_N):
                            nc.tensor.matmul(
                                s_ps[:, ts(m, MM_N)],
                                lhsT=kt_h[:, ts(j, 128)],
                                rhs=qt_h[:, qb * QB + m * MM_N:
                                         qb * QB + (m + 1) * MM_N],
                                start=True, stop=True)
                        e16 = chunks.tile([128, QB], F16, tag="e16")
                        if stage in ("exp", "mult", "pv", "full"):
                            nc.scalar.activation(
                                e16[:], s_ps[:],
                                mybir.ActivationFunctionType.Exp,
                                scale=float(SCALE))
                        ew = chunks.tile([128, QB], F16, tag="ew")
                        if stage in ("mult", "pv", "full"):
                            nc.vector.tensor_mul(ew[:], e16[:],
                                                 wt[j][:, qb * QB:(qb + 1) * QB])
                        if stage in ("pv", "full"):
                            for m in range(QB // MM_N):
                                nc.tensor.matmul(o_ps[:, ts(m, MM_N)],
                                                 lhsT=vp[:, j, :],
                                                 rhs=ew[:, ts(m, MM_N)],
                                                 start=(j == 0),
                                                 stop=(j == NKC - 1))
                    if stage != "full":
                        continue

                    # ---- normalize + transpose + store ----
                    o_sb = outs.tile([D + 1, QB], F32, tag="o_sb")
                    nc.vector.tensor_copy(o_sb[:], o_ps[:])
                    ot = ot_pool.tile([128, QB // 128, D], F32, tag="ot")
                    otz = ot_pool.tile([128, QB // 128], F32, tag="otz")
                    for t in range(QB // 128):
                        nc.tensor.transpose(ot[:, t, :], o_sb[0:D, ts(t, 128)],
                                            ident[0:D, 0:D])
                        nc.tensor.transpose(otz[:, t:t + 1],
                                            o_sb[D:D + 1, ts(t, 128)],
                                            ident[D:D + 1, D:D + 1])
                    rz = outs.tile([128, QB // 128], F32, tag="rz")
                    nc.vector.reciprocal(rz[:], otz[:])
                    o_st = outs.tile([128, QB // 128, D], F32, tag="o_st")
                    nc.vector.tensor_mul(
                        o_st[:], ot[:],
                        rz[:].broadcast_to((128, QB // 128, D)))
                    nc.gpsimd.dma_start(
                        out[h, qb * QB:(qb + 1) * QB, :].rearrange(
                            "(t p) d -> p t d", p=128),
                        o_st[:])
    nc.finalize()
    return nc


_CACHED_NC = None


def _get_program():
    global _CACHED_NC
    if _CACHED_NC is None:
        _CACHED_NC = build_program()
    return _CACHED_NC


def shard_inputs(q, k, v, att_mask, padding_mask):
    """Host-side sharding + layout transforms (transposes only, no math)."""
    attT = np.ascontiguousarray(att_mask[0, 0].T)
    padT = [np.ascontiguousarray(padding_mask[b].T).view(np.uint8)
            for b in range(B)]
    in_maps = []
    for c in range(N_CORES):
        b, hh = divmod(c, 2)
        h0 = hh * HPC
        qc = q[b, h0:h0 + HPC]
        kc = k[b, h0:h0 + HPC]
        in_maps.append({
            "qT": np.ascontiguousarray(qc.transpose(0, 2, 1)),
            "kT": np.ascontiguousarray(kc.transpose(0, 2, 1)),
            "v": np.ascontiguousarray(v[b, h0:h0 + HPC]),
            "attT": attT,
            "padT": padT[b],
        })
    return in_maps


def unshard_output(results):
    out = np.empty((B, H, S, D), dtype=np.float32)
    for c in range(N_CORES):
        b, hh = divmod(c, 2)
        h0 = hh * HPC
        out[b, h0:h0 + HPC] = results[c]["out"]
    return out


def kernel(q, k, v, att_mask, padding_mask):
    q = np.asarray(q, dtype=np.float32)
    k = np.asarray(k, dtype=np.float32)
    v = np.asarray(v, dtype=np.float32)
    att_mask = np.asarray(att_mask, dtype=np.float32)
    padding_mask = np.asarray(padding_mask)
    nc = _get_program()
    in_maps = shard_inputs(q, k, v, att_mask, padding_mask)
    res = run_bass_kernel_spmd(nc, in_maps, list(range(N_CORES)))
    return unshard_output(res.results)


# revision 21
# speedup vs baseline: 3.7810x; 1.2580x over previous
"""Trainium2 Bass kernel for a dense attention block.

Reference computation (per batch b, head h):
    att = (q @ k^T) / sqrt(D) + att_mask          # [S, S]
    att = where(padding_mask[b], -inf, att)
    out = softmax(att, -1) @ v                    # [S, D]

Shapes: q,k,v [4, 16, 2048, 64] f32; att_mask [1,1,2048,2048] f32;
padding_mask [4, 2048, 2048] bool.  Output [4, 16, 2048, 64] f32.

Sharding over 8 cores: core c handles batch b=c//2, heads h in
[8*(c%2), 8*(c%2)+8).  Each core computes 8 full attention heads.

Device algorithm (per core), transposed-score formulation so that the
softmax reduction lands on the PE contraction axis:
  - W^T[k,q] = exp(att_mask[q,k]) * (1 - pad[q,k])   (fp16, SBUF-resident)
    softmax(s+m) == exp(s)*exp(m) / sum(exp(s)*exp(m)); masked entries
    multiply to exactly 0.  No max-subtraction is needed: |scores| <= ~10
    for this distribution, well within fp32/fp16 exp range.
  - per head, per 1024-wide q-block, per 128-wide k-chunk j:
      S^T_j [128k, 1024q] = K_j @ Q^T   (fp32r matmuls, PSUM)
      E_j   = exp(S^T_j / 8)            (ACT, fp16 out)
      EW_j  = E_j * W^T_j               (DVE fp16 2x)
      O^T  += V'_j^T @ EW_j             (fp16 matmul, V' has a ones column
                                         so row 64 of O^T is the denominator)
  - transpose O^T back with PE, multiply by 1/Z, DMA out.
"""

import sys

if "/opt/trn_rl_repo" not in sys.path:
    sys.path.insert(0, "/opt/trn_rl_repo")

import numpy as np

import concourse.bass as bass
import concourse.tile as tile
from concourse import bacc, mybir
from concourse.bass import ts
from concourse.bass_utils import run_bass_kernel_spmd
from concourse.masks import make_identity

F32 = mybir.dt.float32
F32R = mybir.dt.float32r
F16 = mybir.dt.float16
U8 = mybir.dt.uint8

B, H, S, D = 4, 16, 2048, 64
N_CORES = 8
HPC = H // 2          # heads per core
KC = 128              # k-chunk (PSUM partition dim of S^T)
NKC = S // KC         # 16 k-chunks
QB = 1024             # q-block
NQB = S // QB         # q-blocks per head
MM_N = 512            # moving-operand cols per fp32r matmul
SCALE = 1.0 / np.sqrt(D)


def build_program(n_heads=HPC, repeat=1, stage='full'):
    """Build the per-core Bass program (SPMD: identical on all 8 cores).

    repeat>1 re-runs the head loop (timing aid: the device-side cost of one
    pass equals the per-repeat time delta, independent of dispatch latency).
    """
    nc = bacc.Bacc("TRN2", target_bir_lowering=False, debug=False,
                   num_devices=N_CORES)

    qT = nc.declare_dram_parameter("qT", [HPC, D, S], F32, isOutput=False)
    kT = nc.declare_dram_parameter("kT", [HPC, D, S], F32, isOutput=False)
    v_ = nc.declare_dram_parameter("v", [HPC, S, D], F32, isOutput=False)
    attT = nc.declare_dram_parameter("attT", [S, S], F32, isOutput=False)
    padT = nc.declare_dram_parameter("padT", [S, S], U8, isOutput=False)
    out = nc.declare_dram_parameter("out", [HPC, S, D], F32, isOutput=True)

    with tile.TileContext(nc, num_cores=N_CORES) as tc:
        with (
            tc.tile_pool(name="singles", bufs=1) as singles,
            tc.tile_pool(name="wprep", bufs=2) as wprep,
            tc.tile_pool(name="heads", bufs=2) as heads,
            tc.tile_pool(name="chunks", bufs=3) as chunks,
            tc.tile_pool(name="outs", bufs=2) as outs,
            tc.tile_pool(name="ewp", bufs=2) as ewpool,
            tc.tile_pool(name="sp", bufs=2, space="PSUM") as sp_pool,
            tc.tile_pool(name="op", bufs=1, space="PSUM") as op_pool,
            tc.tile_pool(name="otp", bufs=1, space="PSUM") as ot_pool,
        ):
            # ---- constants ----
            ident = singles.tile([128, 128], F32, tag="ident")
            make_identity(nc, ident[:])
            # V' layout [128, NKC, 65] fp16: col 64 of each chunk = 1.0,
            # so row 64 of the PV output is the softmax denominator.
            vp = singles.tile([128, NKC, D + 1], F16, tag="vp")
            nc.gpsimd.memset(vp[:, :, D], 1.0)

            # ---- W^T = exp(attT) * (1 - padT), fp16, SBUF resident ----
            wt = [singles.tile([128, S], F16, name=f"w{j}", tag=f"w{j}")
                  for j in range(NKC)]
            for j in range(NKC):
                att_blk = wprep.tile([128, S], F32, tag="att_blk")
                nc.gpsimd.dma_start(att_blk[:], attT[ts(j, 128), :])
                pad_blk = wprep.tile([128, S], U8, tag="pad_blk")
                nc.gpsimd.dma_start(pad_blk[:], padT[ts(j, 128), :])
                expat = wprep.tile([128, S], F16, tag="expat")
                nc.scalar.activation(expat[:], att_blk[:],
                                     mybir.ActivationFunctionType.Exp)
                # (1 - pad) as fp16 via DVE tensor_scalar (u8 -> f16 convert)
                padf = wprep.tile([128, S], F16, tag="padf")
                nc.vector.tensor_scalar(padf[:], pad_blk[:], -1.0, 1.0,
                                        mybir.AluOpType.mult,
                                        mybir.AluOpType.add)
                nc.vector.tensor_mul(wt[j][:], expat[:], padf[:])

            # ---- main loop over heads ----
            for h_rep in range(n_heads * repeat):
                h = h_rep % n_heads
                kt_h = heads.tile([D, S], F16, tag="kt")
                nc.gpsimd.dma_start(kt_h[:], kT[h])
                qt_h = heads.tile([D, S], F16, tag="qt")
                nc.gpsimd.dma_start(qt_h[:], qT[h])
                nc.gpsimd.dma_start(
                    vp[:, :, 0:D], v_[h].rearrange("(c p) d -> p c d", p=128))

                for qb in range(NQB):
                    o_ps = op_pool.tile([D + 1, QB], F32, tag="op")
                    ew_blk = ewpool.tile([128, NKC, QB], F16, tag="ewb")
                    # phase 1: scores -> exp -> mask-multiply, all chunks
                    for j in range(NKC):
                        s_ps = sp_pool.tile([128, QB], F32, tag="sp")
                        for m in range(QB // MM_N):
                            nc.tensor.matmul(
                                s_ps[:, ts(m, MM_N)],
                                lhsT=kt_h[:, ts(j, 128)],
                                rhs=qt_h[:, qb * QB + m * MM_N:
                                         qb * QB + (m + 1) * MM_N],
                                start=True, stop=True)
                        e16 = chunks.tile([128, QB], F16, tag="e16")
                        if stage in ("exp", "mult", "pv", "full"):
                            nc.scalar.activation(
                                e16[:], s_ps[:],
                                mybir.ActivationFunctionType.Exp,
                                scale=float(SCALE))
                        if stage in ("mult", "pv", "full"):
                            nc.vector.tensor_mul(ew_blk[:, j, :], e16[:],
                                                 wt[j][:, qb * QB:(qb + 1) * QB])
                    # phase 2: PV burst (PE only, no weight-path interleaving)
                    if stage in ("pv", "full"):
                        for j in range(NKC):
                            for m in range(QB // MM_N):
                                nc.tensor.matmul(o_ps[:, ts(m, MM_N)],
                                                 lhsT=vp[:, j, :],
                                                 rhs=ew_blk[:, j, ts(m, MM_N)],
                                                 start=(j == 0),
                                                 stop=(j == NKC - 1))
                    if stage != "full":
                        continue

                    # ---- normalize + transpose + store ----
                    o_sb = outs.tile([D + 1, QB], F32, tag="o_sb")
                    nc.vector.tensor_copy(o_sb[:], o_ps[:])
                    ot = ot_pool.tile([128, QB // 128, D], F32, tag="ot")
                    otz = ot_pool.tile([128, QB // 128], F32, tag="otz")
                    for t in range(QB // 128):
                        nc.tensor.transpose(ot[:, t, :], o_sb[0:D, ts(t, 128)],
                                            ident[0:D, 0:D])
                        nc.tensor.transpose(otz[:, t:t + 1],
                                            o_sb[D:D + 1, ts(t, 128)],
                                            ident[D:D + 1, D:D + 1])
                    rz = outs.tile([128, QB // 128], F32, tag="rz")
                    nc.vector.reciprocal(rz[:], otz[:])
                    o_st = outs.tile([128, QB // 128, D], F32, tag="o_st")
                    nc.vector.tensor_mul(
                        o_st[:], ot[:],
                        rz[:].broadcast_to((128, QB // 128, D)))
                    nc.gpsimd.dma_start(
                        out[h, qb * QB:(qb + 1) * QB, :].rearrange(
                            "(t p) d -> p t d", p=128),
                        o_st[:])
    nc.finalize()
    return nc


_CACHED_NC = None


def _get_program():
    global _CACHED_NC
    if _CACHED_NC is None:
        _CACHED_NC = build_program()
    return _CACHED_NC


def shard_inputs(q, k, v, att_mask, padding_mask):
    """Host-side sharding + layout transforms (transposes only, no math)."""
    attT = np.ascontiguousarray(att_mask[0, 0].T)
    padT = [np.ascontiguousarray(padding_mask[b].T).view(np.uint8)
            for b in range(B)]
    in_maps = []
    for c in range(N_CORES):
        b, hh = divmod(c, 2)
        h0 = hh * HPC
        qc = q[b, h0:h0 + HPC]
        kc = k[b, h0:h0 + HPC]
        in_maps.append({
            "qT": np.ascontiguousarray(qc.transpose(0, 2, 1)),
            "kT": np.ascontiguousarray(kc.transpose(0, 2, 1)),
            "v": np.ascontiguousarray(v[b, h0:h0 + HPC]),
            "attT": attT,
            "padT": padT[b],
        })
    return in_maps


def unshard_output(results):
    out = np.empty((B, H, S, D), dtype=np.float32)
    for c in range(N_CORES):
        b, hh = divmod(c, 2)
        h0 = hh * HPC
        out[b, h0:h0 + HPC] = results[c]["out"]
    return out


def kernel(q, k, v, att_mask, padding_mask):
    q = np.asarray(q, dtype=np.float32)
    k = np.asarray(k, dtype=np.float32)
    v = np.asarray(v, dtype=np.float32)
    att_mask = np.asarray(att_mask, dtype=np.float32)
    padding_mask = np.asarray(padding_mask)
    nc = _get_program()
    in_maps = shard_inputs(q, k, v, att_mask, padding_mask)
    res = run_bass_kernel_spmd(nc, in_maps, list(range(N_CORES)))
    return unshard_output(res.results)


# revision 24
# speedup vs baseline: 5.0490x; 1.3353x over previous
"""Trainium2 Bass kernel for a dense attention block.

Reference computation (per batch b, head h):
    att = (q @ k^T) / sqrt(D) + att_mask          # [S, S]
    att = where(padding_mask[b], -inf, att)
    out = softmax(att, -1) @ v                    # [S, D]

Shapes: q,k,v [4, 16, 2048, 64] f32; att_mask [1,1,2048,2048] f32;
padding_mask [4, 2048, 2048] bool.  Output [4, 16, 2048, 64] f32.

Sharding over 8 cores: core c handles batch b=c//2, heads h in
[8*(c%2), 8*(c%2)+8).  Each core computes 8 full attention heads.

Device algorithm (per core), transposed-score formulation so that the
softmax reduction lands on the PE contraction axis:
  - W^T[k,q] = exp(att_mask[q,k]) * (1 - pad[q,k])   (fp16, SBUF-resident)
    softmax(s+m) == exp(s)*exp(m) / sum(exp(s)*exp(m)); masked entries
    multiply to exactly 0.  No max-subtraction is needed: |scores| <= ~10
    for this distribution, well within fp32/fp16 exp range.
  - per head, per 1024-wide q-block, per 128-wide k-chunk j:
      S^T_j [128k, 1024q] = K_j @ Q^T   (fp32r matmuls, PSUM)
      E_j   = exp(S^T_j / 8)            (ACT, fp16 out)
      EW_j  = E_j * W^T_j               (DVE fp16 2x)
      O^T  += V'_j^T @ EW_j             (fp16 matmul, V' has a ones column
                                         so row 64 of O^T is the denominator)
  - transpose O^T back with PE, multiply by 1/Z, DMA out.
"""

import sys

if "/opt/trn_rl_repo" not in sys.path:
    sys.path.insert(0, "/opt/trn_rl_repo")

import numpy as np

import concourse.bass as bass
import concourse.tile as tile
from concourse import bacc, mybir
from concourse.bass import ts
from concourse.bass_utils import run_bass_kernel_spmd
from concourse.masks import make_identity

F32 = mybir.dt.float32
F32R = mybir.dt.float32r
F16 = mybir.dt.float16
U8 = mybir.dt.uint8

B, H, S, D = 4, 16, 2048, 64
N_CORES = 8
HPC = H // 2          # heads per core
KC = 128              # k-chunk (PSUM partition dim of S^T)
NKC = S // KC         # 16 k-chunks
QB = 1024             # q-block
NQB = S // QB         # q-blocks per head
MM_N = 512            # moving-operand cols per fp32r matmul
SCALE = 1.0 / np.sqrt(D)


def build_program(n_heads=HPC, repeat=1, stage='full'):
    """Build the per-core Bass program (SPMD: identical on all 8 cores).

    repeat>1 re-runs the head loop (timing aid: the device-side cost of one
    pass equals the per-repeat time delta, independent of dispatch latency).
    """
    nc = bacc.Bacc("TRN2", target_bir_lowering=False, debug=False,
                   num_devices=N_CORES)

    qT = nc.declare_dram_parameter("qT", [HPC, D, S], F32, isOutput=False)
    kT = nc.declare_dram_parameter("kT", [HPC, D, S], F32, isOutput=False)
    v_ = nc.declare_dram_parameter("v", [HPC, S, D], F32, isOutput=False)
    attT = nc.declare_dram_parameter("attT", [S, S], F32, isOutput=False)
    padT = nc.declare_dram_parameter("padT", [S, S], U8, isOutput=False)
    out = nc.declare_dram_parameter("out", [HPC, S, D], F32, isOutput=True)

    with tile.TileContext(nc, num_cores=N_CORES) as tc:
        with (
            tc.tile_pool(name="singles", bufs=1) as singles,
            tc.tile_pool(name="wprep", bufs=2) as wprep,
            tc.tile_pool(name="heads", bufs=2) as heads,
            tc.tile_pool(name="chunks", bufs=3) as chunks,
            tc.tile_pool(name="outs", bufs=2) as outs,
            tc.tile_pool(name="ewp", bufs=2) as ewpool,
            tc.tile_pool(name="sp", bufs=2, space="PSUM") as sp_pool,
            tc.tile_pool(name="op", bufs=2, space="PSUM") as op_pool,
        ):
            # ---- constants ----
            ident = singles.tile([128, 128], F32, tag="ident")
            make_identity(nc, ident[:])

            # ---- W^T = exp(attT) * (1 - padT), fp16, SBUF resident ----
            wt = [singles.tile([128, S], F16, name=f"w{j}", tag=f"w{j}")
                  for j in range(NKC)]
            for j in range(NKC):
                att_blk = wprep.tile([128, S], F32, tag="att_blk")
                nc.gpsimd.dma_start(att_blk[:], attT[ts(j, 128), :])
                pad_blk = wprep.tile([128, S], U8, tag="pad_blk")
                nc.gpsimd.dma_start(pad_blk[:], padT[ts(j, 128), :])
                expat = wprep.tile([128, S], F16, tag="expat")
                nc.scalar.activation(expat[:], att_blk[:],
                                     mybir.ActivationFunctionType.Exp)
                # (1 - pad) as fp16 via DVE tensor_scalar (u8 -> f16 convert)
                padf = wprep.tile([128, S], F16, tag="padf")
                nc.vector.tensor_scalar(padf[:], pad_blk[:], -1.0, 1.0,
                                        mybir.AluOpType.mult,
                                        mybir.AluOpType.add)
                nc.vector.tensor_mul(wt[j][:], expat[:], padf[:])

            # ---- main loop: 2-stage software pipeline over (head, q-block)
            # blocks.  Stage A (block i): QK matmuls -> exp -> mask-multiply
            # into a per-block EW buffer.  Stage B (block i-1): PV matmuls,
            # interleaved chunk-by-chunk with stage A so the PE alternates
            # QK/PV and the ACT engine never starves behind a PV burst.
            blocks = [(h_rep % n_heads, qb)
                      for h_rep in range(n_heads * repeat)
                      for qb in range(NQB)]
            prev = None  # (ew_blk, vp_tile, h, qb) of the previous block
            kt_h = qt_h = vp_cur = None
            for i in range(len(blocks) + 1):
                cur = blocks[i] if i < len(blocks) else None
                if cur is not None:
                    h, qb = cur
                    if qb == 0:
                        kt_h = heads.tile([D, S], F16, tag="kt")
                        nc.gpsimd.dma_start(kt_h[:], kT[h])
                        qt_h = heads.tile([D, S], F16, tag="qt")
                        nc.gpsimd.dma_start(qt_h[:], qT[h])
                        # V' is double-buffered (stage B of the last block of
                        # head h runs concurrently with head h+1's loads)
                        vp_cur = heads.tile([128, NKC, D + 1], F16, tag="vp")
                        nc.gpsimd.memset(vp_cur[:, :, D], 1.0)
                        nc.gpsimd.dma_start(
                            vp_cur[:, :, 0:D],
                            v_[h].rearrange("(c p) d -> p c d", p=128))
                    ew_blk = ewpool.tile([128, NKC, QB], F16, tag="ewb")
                o_ps = None
                if prev is not None and stage in ("pv", "full"):
                    o_ps = op_pool.tile([D + 1, QB], F32, tag="op")
                for j in range(NKC):
                    if cur is not None:
                        s_ps = sp_pool.tile([128, QB], F32, tag="sp")
                        for m in range(QB // MM_N):
                            nc.tensor.matmul(
                                s_ps[:, ts(m, MM_N)],
                                lhsT=kt_h[:, ts(j, 128)],
                                rhs=qt_h[:, qb * QB + m * MM_N:
                                         qb * QB + (m + 1) * MM_N],
                                start=True, stop=True)
                        e16 = chunks.tile([128, QB], F16, tag="e16")
                        if stage in ("exp", "mult", "pv", "full"):
                            nc.scalar.activation(
                                e16[:], s_ps[:],
                                mybir.ActivationFunctionType.Exp,
                                scale=float(SCALE))
                        if stage in ("mult", "pv", "full"):
                            nc.vector.tensor_mul(
                                ew_blk[:, j, :], e16[:],
                                wt[j][:, qb * QB:(qb + 1) * QB])
                    if o_ps is not None:
                        p_ew, p_vp, _, _ = prev
                        for m in range(QB // MM_N):
                            nc.tensor.matmul(o_ps[:, ts(m, MM_N)],
                                             lhsT=p_vp[:, j, :],
                                             rhs=p_ew[:, j, ts(m, MM_N)],
                                             start=(j == 0),
                                             stop=(j == NKC - 1))

                # ---- normalize + transpose + store for the previous block
                if o_ps is not None and stage == "full":
                    _, _, ph, pqb = prev
                    o_sb = outs.tile([D + 1, QB], F32, tag="o_sb")
                    nc.vector.tensor_copy(o_sb[:], o_ps[:])
                    # transpose scratch shares the sp pool's PSUM slots
                    # (fits in one: 8*64 + 8 f32 packed into [128, 528];
                    # all transpose outputs stay inside one bank each)
                    otf = sp_pool.tile([128, 528], F32, tag="sp")
                    ot = otf[:, 0:512].rearrange("p (t d) -> p t d", d=D)
                    otz = otf[:, 512:520]
                    for t in range(QB // 128):
                        nc.tensor.transpose(ot[:, t, :], o_sb[0:D, ts(t, 128)],
                                            ident[0:D, 0:D])
                        nc.tensor.transpose(otz[:, t:t + 1],
                                            o_sb[D:D + 1, ts(t, 128)],
                                            ident[D:D + 1, D:D + 1])
                    rz = outs.tile([128, QB // 128], F32, tag="rz")
                    nc.vector.reciprocal(rz[:], otz[:])
                    o_st = outs.tile([128, QB // 128, D], F32, tag="o_st")
                    nc.vector.tensor_mul(
                        o_st[:], ot[:],
                        rz[:].broadcast_to((128, QB // 128, D)))
                    nc.gpsimd.dma_start(
                        out[ph, pqb * QB:(pqb + 1) * QB, :].rearrange(
                            "(t p) d -> p t d", p=128),
                        o_st[:])
                if cur is not None:
                    prev = (ew_blk, vp_cur, h, qb)
    nc.finalize()
    return nc


_CACHED_NC = None


def _get_program():
    global _CACHED_NC
    if _CACHED_NC is None:
        _CACHED_NC = build_program()
    return _CACHED_NC


def shard_inputs(q, k, v, att_mask, padding_mask):
    """Host-side sharding + layout transforms (transposes only, no math)."""
    attT = np.ascontiguousarray(att_mask[0, 0].T)
    padT = [np.ascontiguousarray(padding_mask[b].T).view(np.uint8)
            for b in range(B)]
    in_maps = []
    for c in range(N_CORES):
        b, hh = divmod(c, 2)
        h0 = hh * HPC
        qc = q[b, h0:h0 + HPC]
        kc = k[b, h0:h0 + HPC]
        in_maps.append({
            "qT": np.ascontiguousarray(qc.transpose(0, 2, 1)),
            "kT": np.ascontiguousarray(kc.transpose(0, 2, 1)),
            "v": np.ascontiguousarray(v[b, h0:h0 + HPC]),
            "attT": attT,
            "padT": padT[b],
        })
    return in_maps


def unshard_output(results):
    out = np.empty((B, H, S, D), dtype=np.float32)
    for c in range(N_CORES):
        b, hh = divmod(c, 2)
        h0 = hh * HPC
        out[b, h0:h0 + HPC] = results[c]["out"]
    return out


def kernel(q, k, v, att_mask, padding_mask):
    q = np.asarray(q, dtype=np.float32)
    k = np.asarray(k, dtype=np.float32)
    v = np.asarray(v, dtype=np.float32)
    att_mask = np.asarray(att_mask, dtype=np.float32)
    padding_mask = np.asarray(padding_mask)
    nc = _get_program()
    in_maps = shard_inputs(q, k, v, att_mask, padding_mask)
    res = run_bass_kernel_spmd(nc, in_maps, list(range(N_CORES)))
    return unshard_output(res.results)
